# revision 22
# baseline (speedup 1.0000x reference)
"""BitLinear158 Trainium2 kernel.

Reference computation:
    gamma = mean(|W|)
    Wq    = clip(round(W / (gamma + 1e-5)), -1, 1)      # ternary {-1, 0, +1}
    out   = x @ Wq.T + b                                # x: [8, 4096, 2048]

Sharding: data-parallel over the batch dim (8 batches -> 8 cores). Each core
gets x[i] (host-transposed to k-major so the contraction dim lands on SBUF
partitions with unit-stride DMA), the full W (host-transposed, same reason)
and b, and computes its 4096-token slice of the output. gamma is computed
redundantly per-core from the full W -- no collectives needed.

Device pipeline per core:
  pass 1: stream WT (16 MiB), fused |.| + row-reduce -> partition_all_reduce
          -> gamma -> s = 1/(gamma+eps) (reciprocal + 1 Newton step)
  pass 2: re-stream WT, ternarize via (z>0.5) - (z<-0.5) into resident
          WqT bf16 tiles (exact in bf16)
  main:   for each 128-token tile: DMA xT slice, cast bf16 (ACT),
          16x4 matmuls accumulating into PSUM [128, 2048] (bf16 inputs,
          fp32 accumulate), bias-add on PSUM->SBUF evacuation (DVE),
          DMA out.
"""

from contextlib import ExitStack

import numpy as np

import concourse.bacc as bacc
import concourse.bass as bass
import concourse.mybir as mybir
import concourse.tile as tile
from concourse import library_config
from concourse.bass_isa import ReduceOp
from concourse.bass_utils import run_bass_kernel_spmd

P = 128
B, S, D_IN, D_OUT = 8, 4096, 2048, 2048
N_CORES = 8
TOK = (B * S) // N_CORES          # 4096 tokens per core
KT = D_IN // P                    # 16 k-tiles
TT = TOK // P                     # 32 token tiles
NC_CHUNK = 512                    # matmul moving free dim (1 PSUM bank fp32)
OC = D_OUT // NC_CHUNK            # 4 output chunks
W_ELEMS = D_OUT * D_IN            # 2**22 (power of 2: S/N == S*(1/N) exactly)
EPS = 1e-5

F32 = mybir.dt.float32
BF16 = mybir.dt.bfloat16
MULT = mybir.AluOpType.mult
ADD = mybir.AluOpType.add
IS_GT = mybir.AluOpType.is_gt
IS_GE = mybir.AluOpType.is_ge
AX_X = mybir.AxisListType.X


def build_nc() -> bass.Bass:
    nc = bacc.Bacc(None, target_bir_lowering=False)
    xT = nc.dram_tensor("xT", [D_IN, TOK], F32, kind="ExternalInput")
    WT = nc.dram_tensor("WT", [D_IN, D_OUT], F32, kind="ExternalInput")
    # Per-core slice of WT for the sharded gamma pass (core c gets rows
    # [c*D_IN/8, (c+1)*D_IN/8) -- together they cover all of W).
    GKT = D_IN // N_CORES // P        # gamma k-tiles per core (2)
    Wp1 = nc.dram_tensor("Wp1", [GKT * P, D_OUT], F32, kind="ExternalInput")
    b = nc.dram_tensor("b", [D_OUT], F32, kind="ExternalInput")
    out = nc.dram_tensor("out", [TOK, D_OUT], F32, kind="ExternalOutput")

    with tile.TileContext(nc) as tc, ExitStack() as ctx:
        wpool = ctx.enter_context(tc.tile_pool(name="wpass", bufs=6))
        spool = ctx.enter_context(tc.tile_pool(name="scalars", bufs=1))
        qpool = ctx.enter_context(tc.tile_pool(name="qtmp", bufs=2))
        wqpool = ctx.enter_context(tc.tile_pool(name="wq", bufs=KT))
        xbpool = ctx.enter_context(tc.tile_pool(name="xb", bufs=4))
        opool = ctx.enter_context(tc.tile_pool(name="osb", bufs=2))
        pspool = ctx.enter_context(
            tc.tile_pool(name="psum", bufs=8, space="PSUM")
        )
        drampool = ctx.enter_context(
            tc.tile_pool(name="dram", bufs=2, space="DRAM")
        )

        # Bias replicated to all partitions (partition-broadcast DMA).
        bias_sb = spool.tile([P, D_OUT], F32)
        b_row = b[:].rearrange("(o d) -> o d", o=1)
        nc.sync.dma_start(bias_sb[:], b_row.to_broadcast((P, D_OUT)))

        # ---- pass 1 (sharded): local abs-sum of this core's W slice ----
        gpartials = spool.tile([P, GKT], F32)
        for kt in range(GKT):
            wp = wpool.tile([P, D_OUT], F32, tag="wt", name=f"wp{kt}")
            nc.sync.dma_start(wp[:], Wp1[kt * P : (kt + 1) * P, :])
            nc.vector.reduce_sum(
                gpartials[:, kt : kt + 1],
                wp[:],
                axis=AX_X,
                apply_absolute_value=True,
            )
        colsum = spool.tile([P, 1], F32)
        nc.vector.reduce_sum(colsum[:], gpartials[:], axis=AX_X)

        # AllReduce the [128,1] partials across the 8 cores (DRAM bounce).
        cc_in = drampool.tile([P, 1], F32)
        cc_out = drampool.tile([P, 1], F32)
        nc.sync.dma_start(cc_in[:], colsum[:])
        nc.gpsimd.collective_compute(
            "AllReduce",
            mybir.AluOpType.add,
            replica_groups=[list(range(N_CORES))],
            ins=[cc_in[:].opt()],
            outs=[cc_out[:].opt()],
        )
        colsum_all = spool.tile([P, 1], F32)
        nc.sync.dma_start(colsum_all[:], cc_out[:])

        # Partition reduce + broadcast in one PE op: ones.T @ colsum puts
        # sum over partitions on every partition.
        ones_sq = spool.tile([P, P], F32)
        nc.vector.memset(ones_sq[:], 1.0)
        total_ps = pspool.tile([P, NC_CHUNK], F32, tag="ps")
        nc.tensor.matmul(
            total_ps[:, 0:1], ones_sq[:], colsum_all[:], start=True, stop=True
        )
        total = spool.tile([P, 1], F32)
        nc.vector.tensor_copy(total[:], total_ps[:, 0:1])

        # Quantization thresholds: W > thr  <=>  W/(gamma+eps) > 0.5.
        # Comparing W directly against +-0.5*(gamma+eps) skips the
        # reciprocal entirely.
        geps = spool.tile([P, 1], F32)
        nc.vector.tensor_scalar(geps[:], total[:], 1.0 / W_ELEMS, EPS, MULT, ADD)
        thr = spool.tile([P, 1], F32)
        nc.vector.tensor_scalar_mul(thr[:], geps[:], 0.5)
        negthr = spool.tile([P, 1], F32)
        nc.vector.tensor_scalar_mul(negthr[:], geps[:], -0.5)

        # ---- pass 2: WqT = (W > thr) + (W >= -thr) - 1 in {-1, 0, +1} ----
        # Two DVE ops per tile: a = (W > thr) - 1 in {-1, 0}, then
        # wq = (W >= -thr) + a.
        wq_tiles = []
        for kt in range(KT):
            wt = wpool.tile([P, D_OUT], F32, tag="wt")
            nc.sync.dma_start(wt[:], WT[kt * P : (kt + 1) * P, :])
            ga = qpool.tile([P, D_OUT], BF16, tag="q")
            nc.vector.tensor_scalar(ga[:], wt[:], thr[:], -1.0, IS_GT, ADD)
            wq = wqpool.tile([P, D_OUT], BF16, tag="wq")
            nc.vector.scalar_tensor_tensor(
                wq[:], wt[:], negthr[:], ga[:], IS_GE, ADD
            )
            wq_tiles.append(wq)

        # ---- main: out[t, :] = x[t, :] @ WqT + b ----
        # Epochs of 2 token-tiles x 4 output chunks = 8 concurrent [128,512]
        # PSUM accumulation groups (all 8 banks). k-major MM order means one
        # arriving WqT k-tile enables 8 matmuls, so PE ramps while the
        # quantize pipeline is still filling.
        xT_v = xT.rearrange("(a p) t -> p a t", p=P)  # [128, KT, TOK]
        TPE = 2  # token tiles per epoch
        for ep in range(TT // TPE):
            xbs = []
            for i in range(TPE):
                tt = ep * TPE + i
                # SWDGE DMA casts fp32 -> bf16 inline (RNE): activations land
                # in SBUF already in matmul dtype, no compute-engine work.
                xb = xbpool.tile([P, KT, P], BF16, tag="xb")
                nc.gpsimd.dma_start(xb[:], xT_v[:, :, tt * P : (tt + 1) * P])
                xbs.append(xb)

            groups = [(i, oc) for i in range(TPE) for oc in range(OC)]
            pss = [
                pspool.tile([P, NC_CHUNK], F32, tag="ps", name=f"ps{g}")
                for g in range(len(groups))
            ]
            for kt in range(KT):
                for g, (i, oc) in enumerate(groups):
                    nc.tensor.matmul(
                        pss[g][:],
                        xbs[i][:, kt, :],
                        wq_tiles[kt][:, oc * NC_CHUNK : (oc + 1) * NC_CHUNK],
                        start=(kt == 0),
                        stop=(kt == KT - 1),
                    )

            for i in range(TPE):
                tt = ep * TPE + i
                osb = opool.tile([P, D_OUT], F32, tag="osb")
                for oc in range(OC):
                    nc.vector.tensor_add(
                        osb[:, oc * NC_CHUNK : (oc + 1) * NC_CHUNK],
                        pss[i * OC + oc][:],
                        bias_sb[:, oc * NC_CHUNK : (oc + 1) * NC_CHUNK],
                    )
                nc.sync.dma_start(out[tt * P : (tt + 1) * P, :], osb[:])

    nc.finalize()
    return nc


_NC_CACHE: list = []


def _get_nc() -> bass.Bass:
    if not _NC_CACHE:
        _NC_CACHE.append(build_nc())
    return _NC_CACHE[0]


def make_in_maps(x: np.ndarray, W: np.ndarray, b: np.ndarray):
    x = np.asarray(x, dtype=np.float32).reshape(N_CORES, TOK, D_IN)
    W = np.asarray(W, dtype=np.float32)
    b = np.asarray(b, dtype=np.float32)
    WT = np.ascontiguousarray(W.T)
    return [
        {
            "xT": np.ascontiguousarray(x[c].T),
            "WT": WT,
            "Wp1": WT[c * (D_IN // N_CORES) : (c + 1) * (D_IN // N_CORES), :],
            "b": b,
        }
        for c in range(N_CORES)
    ]


def run(x, W, b, **spmd_kwargs):
    """Run the SPMD kernel; returns (full_output, BassKernelResults)."""
    nc = _get_nc()
    in_maps = make_in_maps(x, W, b)
    res = run_bass_kernel_spmd(nc, in_maps, list(range(N_CORES)), **spmd_kwargs)
    out = np.stack([res.results[c]["out"] for c in range(N_CORES)], axis=0)
    return out.reshape(B, S, D_OUT), res


def kernel(x, W, b):
    out, _ = run(x, W, b)
    return out
